# revision 3
# baseline (speedup 1.0000x reference)
"""Trainium2 Bass kernel for nn_Base_Filter (depthwise 7x7 conv + weight-norm +
1x1 projection residual + leaky-decay-relu), sharded over K=1024 channels
across 8 NeuronCores.

Math (folded on host):
  y      = x*(1+w_p) + b_p                       (per-channel affine)
  w_eff  = g * v / ||v||_F                       (weight norm, per channel)
  z      = depthwise_conv7x7_valid(y, w_eff)
  out    = where(z>0, 0.9*z, 0.01*z)

Linearity fold: z = conv(x, w_eff)*(1+w_p) + b_p*sum(w_eff), so with
  w2 = 0.9*(1+w_p)*w_eff,  c2 = 0.9*b_p*sum(w_eff)
we get  out = lrelu(conv(x, w2) + c2, alpha=1/90)  elementwise.

Device kernel (per core, 128 channels on 128 partitions):
  - 49-tap accumulation split between TensorE (diagonal-lhsT matmuls
    accumulating in PSUM) and VectorE (tensor_scalar / scalar_tensor_tensor
    with per-partition scalar weights).
  - ScalarE applies Lrelu(+bias c2) while evacuating to SBUF.
  - All DMA is contiguous per partition (host pre-transposes x to
    channel-major [1024, 256, 256] and post-transposes the output).
"""

import os
import numpy as np

A = 256
B = 256
R = 32
C = 32
K = 1024
KS = 7
NCORES = 8
P = 128          # channels per core = partitions
AO = A - KS + 1  # 250
BO = B - KS + 1  # 250

H = 26           # output rows per strip (last strip: 16)
TR = 2           # output rows per PSUM tile -> N = TR*BO = 500 <= 512 fp32

# Tap split between TensorE and VectorE (49 taps total), tunable via env.
N_PE = int(os.environ.get("KRN_N_PE", "36"))
N_DVE = KS * KS - N_PE

_COMPILED = {}
LAST_RESULTS = None  # BassKernelResults of the most recent run (for test.py)


def _build_nc():
    import concourse.bacc as bacc
    import concourse.mybir as mybir
    import concourse.tile as tile

    f32 = mybir.dt.float32
    nc = bacc.Bacc("TRN2", target_bir_lowering=False, debug=False, num_devices=NCORES)

    x_d = nc.declare_dram_parameter("x", [P, A, B], f32, isOutput=False)
    dg_d = nc.declare_dram_parameter("dg", [P, max(N_PE, 1), P], f32, isOutput=False)
    wv_d = nc.declare_dram_parameter("wv", [P, max(N_DVE, 1)], f32, isOutput=False)
    c2_d = nc.declare_dram_parameter("c2", [P, 1], f32, isOutput=False)
    out_d = nc.declare_dram_parameter("out", [P, AO, BO], f32, isOutput=True)

    taps = [(di, dj) for di in range(KS) for dj in range(KS)]
    pe_taps = taps[:N_PE]
    dve_taps = taps[N_PE:]

    with tile.TileContext(nc) as tc:
        from contextlib import ExitStack

        with ExitStack() as ctx:
            const = ctx.enter_context(tc.tile_pool(name="const", bufs=1))
            xpool = ctx.enter_context(tc.tile_pool(name="x", bufs=2))
            opool = ctx.enter_context(tc.tile_pool(name="o", bufs=2))
            apool = ctx.enter_context(tc.tile_pool(name="acc", bufs=3))
            prepool = ctx.enter_context(tc.tile_pool(name="pre", bufs=3))
            ppool = ctx.enter_context(tc.tile_pool(name="ps", bufs=4, space="PSUM"))

            dg_sb = const.tile([P, max(N_PE, 1), P], f32)
            nc.sync.dma_start(dg_sb[:], dg_d[:])
            wv_sb = const.tile([P, max(N_DVE, 1)], f32)
            nc.sync.dma_start(wv_sb[:], wv_d[:])
            c2_sb = const.tile([P, 1], f32)
            nc.sync.dma_start(c2_sb[:], c2_d[:])

            row0 = 0
            while row0 < AO:
                rows = min(H, AO - row0)
                in_rows = rows + KS - 1
                xs = xpool.tile([P, in_rows, B], f32, tag="xs")
                nc.sync.dma_start(xs[:], x_d[:, row0 : row0 + in_rows, :])
                outs = opool.tile([P, rows, BO], f32, tag="outs")

                o0 = 0
                while o0 < rows:
                    tr = min(TR, rows - o0)
                    ps = ppool.tile([P, tr, BO], f32, tag="ps")
                    for i, (di, dj) in enumerate(pe_taps):
                        rhs = xs[:, o0 + di : o0 + di + tr, dj : dj + BO]
                        nc.tensor.matmul(
                            ps[:],
                            dg_sb[:, i, :],
                            rhs,
                            start=(i == 0),
                            stop=(i == len(pe_taps) - 1),
                        )
                    acc = apool.tile([P, tr, BO], f32, tag="acc")
                    for j, (di, dj) in enumerate(dve_taps):
                        rhs = xs[:, o0 + di : o0 + di + tr, dj : dj + BO]
                        if j == 0:
                            nc.vector.tensor_scalar(
                                acc[:], rhs, wv_sb[:, 0:1], None, mybir.AluOpType.mult
                            )
                        else:
                            nc.vector.scalar_tensor_tensor(
                                acc[:],
                                rhs,
                                wv_sb[:, j : j + 1],
                                acc[:],
                                mybir.AluOpType.mult,
                                mybir.AluOpType.add,
                            )
                    pre = prepool.tile([P, tr, BO], f32, tag="pre")
                    if N_DVE > 0:
                        # pre = (ps * 1.0) + acc
                        nc.vector.scalar_tensor_tensor(
                            pre[:],
                            ps[:],
                            1.0,
                            acc[:],
                            mybir.AluOpType.mult,
                            mybir.AluOpType.add,
                        )
                        src = pre
                    else:
                        src = ps
                    # out = lrelu(src + c2), alpha = 0.01/0.9
                    nc.scalar.activation(
                        outs[:, o0 : o0 + tr, :],
                        src[:],
                        mybir.ActivationFunctionType.Lrelu,
                        bias=c2_sb[:, 0:1],
                        scale=1.0,
                        alpha=0.01 / 0.9,
                    )
                    o0 += tr

                nc.sync.dma_start(out_d[:, row0 : row0 + rows, :], outs[:])
                row0 += rows

    nc.compile()
    return nc


def _prep_weights(w_p, b_p, v, g):
    v = v.astype(np.float32)
    v_norm = np.sqrt((v * v).sum(axis=(1, 2), keepdims=True))
    w_eff = g[:, None, None].astype(np.float32) * v / v_norm          # [K,7,7]
    w2 = 0.9 * (1.0 + w_p)[:, None, None].astype(np.float32) * w_eff  # [K,7,7]
    c2 = (0.9 * b_p.astype(np.float32) * w_eff.sum(axis=(1, 2)))      # [K]
    return w2.astype(np.float32), c2.astype(np.float32)


def kernel(x, w_p, b_p, v, g):
    global LAST_RESULTS
    from concourse.bass_utils import run_bass_kernel_spmd

    x = np.asarray(x, dtype=np.float32)
    w2, c2 = _prep_weights(
        np.asarray(w_p, np.float32),
        np.asarray(b_p, np.float32),
        np.asarray(v, np.float32),
        np.asarray(g, np.float32),
    )

    # channel-major x: [K, A, B], k = r*C + c (matches reference's kernel_index)
    x_t = np.ascontiguousarray(x.transpose(2, 3, 0, 1).reshape(K, A, B))

    taps = [(di, dj) for di in range(KS) for dj in range(KS)]
    in_maps = []
    ar = np.arange(P)
    for core in range(NCORES):
        sl = slice(core * P, (core + 1) * P)
        w2c = w2[sl]  # [P,7,7]
        dg = np.zeros((max(N_PE, 1), P, P), dtype=np.float32)
        for i, (di, dj) in enumerate(taps[:N_PE]):
            dg[i, ar, ar] = w2c[:, di, dj]
        # SBUF layout [P, N_PE, P]: dg_sb[p, t, m] = dg[t, p, m]
        dg_sb = np.ascontiguousarray(dg.transpose(1, 0, 2))
        wv = np.zeros((P, max(N_DVE, 1)), dtype=np.float32)
        for j, (di, dj) in enumerate(taps[N_PE:]):
            wv[:, j] = w2c[:, di, dj]
        in_maps.append(
            {
                "x": np.ascontiguousarray(x_t[sl]),
                "dg": dg_sb,
                "wv": wv,
                "c2": np.ascontiguousarray(c2[sl][:, None]),
            }
        )

    key = ("v1", N_PE)
    if key not in _COMPILED:
        _COMPILED[key] = _build_nc()
    nc = _COMPILED[key]

    trace = os.environ.get("KRN_TRACE", "0") == "1"
    res = run_bass_kernel_spmd(nc, in_maps, list(range(NCORES)), trace=trace)
    LAST_RESULTS = res

    out_full = np.empty((K, AO, BO), dtype=np.float32)
    for core in range(NCORES):
        out_full[core * P : (core + 1) * P] = res.results[core]["out"]

    # [K, AO, BO] -> [AO, BO, R, C]
    return np.ascontiguousarray(
        out_full.reshape(R, C, AO, BO).transpose(2, 3, 0, 1)
    )


if __name__ == "__main__":
    rng = np.random.default_rng(0)
    xs = rng.standard_normal((A, B, R, C), dtype=np.float32)
    out = kernel(
        xs,
        rng.standard_normal(K).astype(np.float32) * 0.1,
        rng.standard_normal(K).astype(np.float32) * 0.1,
        rng.standard_normal((K, KS, KS)).astype(np.float32),
        rng.standard_normal(K).astype(np.float32),
    )
    print(out.shape, out.dtype)


# revision 6
# speedup vs baseline: 1.3830x; 1.3830x over previous
"""Trainium2 Bass kernel for nn_Base_Filter (depthwise 7x7 conv + weight-norm +
1x1 projection residual + leaky-decay-relu), sharded over K=1024 channels
across 8 NeuronCores.

Math (folded on host):
  y      = x*(1+w_p) + b_p                       (per-channel affine)
  w_eff  = g * v / ||v||_F                       (weight norm, per channel)
  z      = depthwise_conv7x7_valid(y, w_eff)
  out    = where(z>0, 0.9*z, 0.01*z)

Linearity fold: z = conv(x, w_eff)*(1+w_p) + b_p*sum(w_eff), so with
  w2 = 0.9*(1+w_p)*w_eff,  c2 = 0.9*b_p*sum(w_eff)
we get  out = lrelu(conv(x, w2) + c2, alpha=1/90)  elementwise.

Device kernel (per core, 128 channels on 128 partitions):
  - 49-tap accumulation split between TensorE (diagonal-lhsT matmuls
    accumulating in PSUM) and VectorE (tensor_scalar / scalar_tensor_tensor
    with per-partition scalar weights).
  - ScalarE applies Lrelu(+bias c2) while evacuating to SBUF.
  - All DMA is contiguous per partition (host pre-transposes x to
    channel-major [1024, 256, 256] and post-transposes the output).
"""

import os
import numpy as np

A = 256
B = 256
R = 32
C = 32
K = 1024
KS = 7
NCORES = 8
P = 128          # channels per core = partitions
AO = A - KS + 1  # 250
BO = B - KS + 1  # 250

H = 26           # output rows per strip (last strip: 16)
TR = 2           # output rows per PSUM tile -> N = TR*BO = 500 <= 512 fp32

# Tap split between TensorE and VectorE (49 taps total), tunable via env.
N_PE = int(os.environ.get("KRN_N_PE", "36"))
N_DVE = KS * KS - N_PE

_COMPILED = {}
LAST_RESULTS = None  # BassKernelResults of the most recent run (for test.py)


def _build_nc():
    import concourse.bacc as bacc
    import concourse.mybir as mybir
    import concourse.tile as tile

    f32 = mybir.dt.float32
    nc = bacc.Bacc("TRN2", target_bir_lowering=False, debug=False, num_devices=NCORES)

    f32r = mybir.dt.float32r
    x_d = nc.declare_dram_parameter("x", [P, A, B], f32r, isOutput=False)
    dg_d = nc.declare_dram_parameter("dg", [P, max(N_PE, 1), P], f32r, isOutput=False)
    wv_d = nc.declare_dram_parameter("wv", [P, max(N_DVE, 1)], f32, isOutput=False)
    c2_d = nc.declare_dram_parameter("c2", [P, 1], f32, isOutput=False)
    out_d = nc.declare_dram_parameter("out", [P, AO, BO], f32, isOutput=True)

    taps = [(di, dj) for di in range(KS) for dj in range(KS)]
    pe_taps = taps[:N_PE]
    dve_taps = taps[N_PE:]

    with tile.TileContext(nc) as tc:
        from contextlib import ExitStack

        with ExitStack() as ctx:
            const = ctx.enter_context(tc.tile_pool(name="const", bufs=1))
            xpool = ctx.enter_context(tc.tile_pool(name="x", bufs=2))
            opool = ctx.enter_context(tc.tile_pool(name="o", bufs=2))
            apool = ctx.enter_context(tc.tile_pool(name="acc", bufs=3))
            prepool = ctx.enter_context(tc.tile_pool(name="pre", bufs=3))
            ppool = ctx.enter_context(tc.tile_pool(name="ps", bufs=4, space="PSUM"))

            dg_sb = const.tile([P, max(N_PE, 1), P], f32r)
            nc.sync.dma_start(dg_sb[:], dg_d[:])
            wv_sb = const.tile([P, max(N_DVE, 1)], f32)
            nc.sync.dma_start(wv_sb[:], wv_d[:])
            c2_sb = const.tile([P, 1], f32)
            nc.sync.dma_start(c2_sb[:], c2_d[:])

            row0 = 0
            while row0 < AO:
                rows = min(H, AO - row0)
                in_rows = rows + KS - 1
                xs = xpool.tile([P, in_rows, B], f32r, tag="xs")
                nc.sync.dma_start(xs[:], x_d[:, row0 : row0 + in_rows, :])
                outs = opool.tile([P, rows, BO], f32, tag="outs")

                o0 = 0
                while o0 < rows:
                    tr = min(TR, rows - o0)
                    ps = ppool.tile([P, tr, BO], f32, tag="ps")
                    for i, (di, dj) in enumerate(pe_taps):
                        rhs = xs[:, o0 + di : o0 + di + tr, dj : dj + BO]
                        # float32r: full-rate (1 cycle/row) fp32 matmul
                        nc.tensor.matmul(
                            ps[:],
                            dg_sb[:, i, :],
                            rhs,
                            start=(i == 0),
                            stop=(i == len(pe_taps) - 1),
                        )
                    acc = apool.tile([P, tr, BO], f32, tag="acc")
                    for j, (di, dj) in enumerate(dve_taps):
                        rhs = xs[:, o0 + di : o0 + di + tr, dj : dj + BO].bitcast(f32)
                        if j == 0:
                            nc.vector.tensor_scalar(
                                acc[:], rhs, wv_sb[:, 0:1], None, mybir.AluOpType.mult
                            )
                        else:
                            nc.vector.scalar_tensor_tensor(
                                acc[:],
                                rhs,
                                wv_sb[:, j : j + 1],
                                acc[:],
                                mybir.AluOpType.mult,
                                mybir.AluOpType.add,
                            )
                    pre = prepool.tile([P, tr, BO], f32, tag="pre")
                    if N_DVE > 0:
                        # pre = (ps * 1.0) + acc
                        nc.vector.scalar_tensor_tensor(
                            pre[:],
                            ps[:],
                            1.0,
                            acc[:],
                            mybir.AluOpType.mult,
                            mybir.AluOpType.add,
                        )
                        src = pre
                    else:
                        src = ps
                    # out = lrelu(src + c2), alpha = 0.01/0.9
                    nc.scalar.activation(
                        outs[:, o0 : o0 + tr, :],
                        src[:],
                        mybir.ActivationFunctionType.Lrelu,
                        bias=c2_sb[:, 0:1],
                        scale=1.0,
                        alpha=0.01 / 0.9,
                    )
                    o0 += tr

                # scalar = second HWDGE ring; keeps output DMA off the
                # input-DMA ring
                nc.scalar.dma_start(out_d[:, row0 : row0 + rows, :], outs[:])
                row0 += rows

    nc.compile()
    return nc


def _prep_weights(w_p, b_p, v, g):
    v = v.astype(np.float32)
    v_norm = np.sqrt((v * v).sum(axis=(1, 2), keepdims=True))
    w_eff = g[:, None, None].astype(np.float32) * v / v_norm          # [K,7,7]
    w2 = 0.9 * (1.0 + w_p)[:, None, None].astype(np.float32) * w_eff  # [K,7,7]
    c2 = (0.9 * b_p.astype(np.float32) * w_eff.sum(axis=(1, 2)))      # [K]
    return w2.astype(np.float32), c2.astype(np.float32)


def kernel(x, w_p, b_p, v, g):
    global LAST_RESULTS
    from concourse.bass_utils import run_bass_kernel_spmd

    x = np.asarray(x, dtype=np.float32)
    w2, c2 = _prep_weights(
        np.asarray(w_p, np.float32),
        np.asarray(b_p, np.float32),
        np.asarray(v, np.float32),
        np.asarray(g, np.float32),
    )

    # channel-major x: [K, A, B], k = r*C + c (matches reference's kernel_index)
    x_t = np.ascontiguousarray(x.transpose(2, 3, 0, 1).reshape(K, A, B))

    taps = [(di, dj) for di in range(KS) for dj in range(KS)]
    in_maps = []
    ar = np.arange(P)
    for core in range(NCORES):
        sl = slice(core * P, (core + 1) * P)
        w2c = w2[sl]  # [P,7,7]
        dg = np.zeros((max(N_PE, 1), P, P), dtype=np.float32)
        for i, (di, dj) in enumerate(taps[:N_PE]):
            dg[i, ar, ar] = w2c[:, di, dj]
        # SBUF layout [P, N_PE, P]: dg_sb[p, t, m] = dg[t, p, m]
        dg_sb = np.ascontiguousarray(dg.transpose(1, 0, 2))
        wv = np.zeros((P, max(N_DVE, 1)), dtype=np.float32)
        for j, (di, dj) in enumerate(taps[N_PE:]):
            wv[:, j] = w2c[:, di, dj]
        in_maps.append(
            {
                "x": np.ascontiguousarray(x_t[sl]),
                "dg": dg_sb,
                "wv": wv,
                "c2": np.ascontiguousarray(c2[sl][:, None]),
            }
        )

    key = ("v1", N_PE)
    if key not in _COMPILED:
        _COMPILED[key] = _build_nc()
    nc = _COMPILED[key]

    trace = os.environ.get("KRN_TRACE", "0") == "1"
    res = run_bass_kernel_spmd(nc, in_maps, list(range(NCORES)), trace=trace)
    LAST_RESULTS = res

    out_full = np.empty((K, AO, BO), dtype=np.float32)
    for core in range(NCORES):
        out_full[core * P : (core + 1) * P] = res.results[core]["out"]

    # [K, AO, BO] -> [AO, BO, R, C]
    return np.ascontiguousarray(
        out_full.reshape(R, C, AO, BO).transpose(2, 3, 0, 1)
    )


if __name__ == "__main__":
    rng = np.random.default_rng(0)
    xs = rng.standard_normal((A, B, R, C), dtype=np.float32)
    out = kernel(
        xs,
        rng.standard_normal(K).astype(np.float32) * 0.1,
        rng.standard_normal(K).astype(np.float32) * 0.1,
        rng.standard_normal((K, KS, KS)).astype(np.float32),
        rng.standard_normal(K).astype(np.float32),
    )
    print(out.shape, out.dtype)


# revision 11
# speedup vs baseline: 1.7070x; 1.2343x over previous
"""Trainium2 Bass kernel for nn_Base_Filter (depthwise 7x7 conv + weight-norm +
1x1 projection residual + leaky-decay-relu), sharded over K=1024 channels
across 8 NeuronCores.

Math (folded on host):
  y      = x*(1+w_p) + b_p                       (per-channel affine)
  w_eff  = g * v / ||v||_F                       (weight norm, per channel)
  z      = depthwise_conv7x7_valid(y, w_eff)
  out    = where(z>0, 0.9*z, 0.01*z)

Linearity fold: z = conv(x, w_eff)*(1+w_p) + b_p*sum(w_eff), so with
  w2 = 0.9*(1+w_p)*w_eff,  c2 = 0.9*b_p*sum(w_eff)
we get  out = lrelu(conv(x, w2) + c2, alpha=1/90)  elementwise.

Device kernel (per core, 128 channels on 128 partitions):
  - 49-tap accumulation split between TensorE (diagonal-lhsT matmuls
    accumulating in PSUM) and VectorE (tensor_scalar / scalar_tensor_tensor
    with per-partition scalar weights).
  - ScalarE applies Lrelu(+bias c2) while evacuating to SBUF.
  - All DMA is contiguous per partition (host pre-transposes x to
    channel-major [1024, 256, 256] and post-transposes the output).
"""

import os
import numpy as np

A = 256
B = 256
R = 32
C = 32
K = 1024
KS = 7
NCORES = 8
P = 128          # channels per core = partitions
AO = A - KS + 1  # 250
BO = B - KS + 1  # 250

H = 26           # output rows per strip (last strip: 16)
TR = 4           # output rows per PSUM tile (2 banks; matmuls go per 2-row half)
BP = 256         # padded row pitch in PSUM so each 2-row half sits in one bank

# Tap split between TensorE and VectorE (49 taps total), tunable via env.
N_PE = int(os.environ.get("KRN_N_PE", "36"))
N_DVE = KS * KS - N_PE

_COMPILED = {}
LAST_RESULTS = None  # BassKernelResults of the most recent run (for test.py)


def _build_nc():
    import concourse.bacc as bacc
    import concourse.mybir as mybir
    import concourse.tile as tile

    f32 = mybir.dt.float32
    nc = bacc.Bacc("TRN2", target_bir_lowering=False, debug=False, num_devices=NCORES)

    f32r = mybir.dt.float32r
    x_d = nc.declare_dram_parameter("x", [P, A, B], f32r, isOutput=False)
    dg_d = nc.declare_dram_parameter("dg", [P, max(N_PE, 1), P], f32r, isOutput=False)
    wv_d = nc.declare_dram_parameter("wv", [P, max(N_DVE, 1)], f32, isOutput=False)
    c2_d = nc.declare_dram_parameter("c2", [P, 1], f32, isOutput=False)
    out_d = nc.declare_dram_parameter("out", [P, AO, BO], f32, isOutput=True)

    taps = [(di, dj) for di in range(KS) for dj in range(KS)]
    pe_taps = taps[:N_PE]
    dve_taps = taps[N_PE:]

    with tile.TileContext(nc) as tc:
        from contextlib import ExitStack

        with ExitStack() as ctx:
            const = ctx.enter_context(tc.tile_pool(name="const", bufs=1))
            xpool = ctx.enter_context(tc.tile_pool(name="x", bufs=2))
            opool = ctx.enter_context(tc.tile_pool(name="o", bufs=2))
            apool = ctx.enter_context(tc.tile_pool(name="acc", bufs=3))
            prepool = ctx.enter_context(tc.tile_pool(name="pre", bufs=3))
            ppool = ctx.enter_context(tc.tile_pool(name="ps", bufs=4, space="PSUM"))

            dg_sb = const.tile([P, max(N_PE, 1), P], f32r)
            nc.sync.dma_start(dg_sb[:], dg_d[:])
            wv_sb = const.tile([P, max(N_DVE, 1)], f32)
            nc.sync.dma_start(wv_sb[:], wv_d[:])
            c2_sb = const.tile([P, 1], f32)
            nc.sync.dma_start(c2_sb[:], c2_d[:])

            row0 = 0
            while row0 < AO:
                rows = min(H, AO - row0)
                in_rows = rows + KS - 1
                xs = xpool.tile([P, in_rows, B], f32r, tag="xs")
                nc.sync.dma_start(xs[:], x_d[:, row0 : row0 + in_rows, :])
                outs = opool.tile([P, rows, BO], f32, tag="outs")

                o0 = 0
                while o0 < rows:
                    tr = min(TR, rows - o0)
                    ps = ppool.tile([P, TR, BP], f32, tag="ps")
                    for h in range(0, tr, 2):
                        hr = min(2, tr - h)
                        out_ap = ps[:, h : h + hr, 0:BO]
                        for i, (di, dj) in enumerate(pe_taps):
                            rhs = xs[:, o0 + h + di : o0 + h + di + hr, dj : dj + BO]
                            # float32r: full-rate (1 cycle/row) fp32 matmul
                            nc.tensor.matmul(
                                out_ap,
                                dg_sb[:, i, :],
                                rhs,
                                start=(i == 0),
                                stop=(i == len(pe_taps) - 1),
                            )
                    ps_ap = ps[:, 0:tr, 0:BO]
                    if N_DVE > 0:
                        acc = apool.tile([P, TR, BO], f32, tag="acc")
                        acc_ap = acc[:, 0:tr, :]
                        for j, (di, dj) in enumerate(dve_taps):
                            rhs = xs[
                                :, o0 + di : o0 + di + tr, dj : dj + BO
                            ].bitcast(f32)
                            # first tap folds the PSUM total into the chain
                            prev = ps_ap if j == 0 else acc_ap
                            nc.vector.scalar_tensor_tensor(
                                acc_ap,
                                rhs,
                                wv_sb[:, j : j + 1],
                                prev,
                                mybir.AluOpType.mult,
                                mybir.AluOpType.add,
                            )
                        src = acc_ap
                    else:
                        src = ps_ap
                    # out = lrelu(src + c2), alpha = 0.01/0.9
                    nc.scalar.activation(
                        outs[:, o0 : o0 + tr, :],
                        src,
                        mybir.ActivationFunctionType.Lrelu,
                        bias=c2_sb[:, 0:1],
                        scale=1.0,
                        alpha=0.01 / 0.9,
                    )
                    o0 += tr

                # scalar = second HWDGE ring; keeps output DMA off the
                # input-DMA ring
                nc.scalar.dma_start(out_d[:, row0 : row0 + rows, :], outs[:])
                row0 += rows

    nc.compile()
    return nc


def _prep_weights(w_p, b_p, v, g):
    v = v.astype(np.float32)
    v_norm = np.sqrt((v * v).sum(axis=(1, 2), keepdims=True))
    w_eff = g[:, None, None].astype(np.float32) * v / v_norm          # [K,7,7]
    w2 = 0.9 * (1.0 + w_p)[:, None, None].astype(np.float32) * w_eff  # [K,7,7]
    c2 = (0.9 * b_p.astype(np.float32) * w_eff.sum(axis=(1, 2)))      # [K]
    return w2.astype(np.float32), c2.astype(np.float32)


def kernel(x, w_p, b_p, v, g):
    global LAST_RESULTS
    from concourse.bass_utils import run_bass_kernel_spmd

    x = np.asarray(x, dtype=np.float32)
    w2, c2 = _prep_weights(
        np.asarray(w_p, np.float32),
        np.asarray(b_p, np.float32),
        np.asarray(v, np.float32),
        np.asarray(g, np.float32),
    )

    # channel-major x: [K, A, B], k = r*C + c (matches reference's kernel_index)
    x_t = np.ascontiguousarray(x.transpose(2, 3, 0, 1).reshape(K, A, B))

    taps = [(di, dj) for di in range(KS) for dj in range(KS)]
    in_maps = []
    ar = np.arange(P)
    for core in range(NCORES):
        sl = slice(core * P, (core + 1) * P)
        w2c = w2[sl]  # [P,7,7]
        dg = np.zeros((max(N_PE, 1), P, P), dtype=np.float32)
        for i, (di, dj) in enumerate(taps[:N_PE]):
            dg[i, ar, ar] = w2c[:, di, dj]
        # SBUF layout [P, N_PE, P]: dg_sb[p, t, m] = dg[t, p, m]
        dg_sb = np.ascontiguousarray(dg.transpose(1, 0, 2))
        wv = np.zeros((P, max(N_DVE, 1)), dtype=np.float32)
        for j, (di, dj) in enumerate(taps[N_PE:]):
            wv[:, j] = w2c[:, di, dj]
        in_maps.append(
            {
                "x": np.ascontiguousarray(x_t[sl]),
                "dg": dg_sb,
                "wv": wv,
                "c2": np.ascontiguousarray(c2[sl][:, None]),
            }
        )

    key = ("v1", N_PE)
    if key not in _COMPILED:
        _COMPILED[key] = _build_nc()
    nc = _COMPILED[key]

    trace = os.environ.get("KRN_TRACE", "0") == "1"
    res = run_bass_kernel_spmd(nc, in_maps, list(range(NCORES)), trace=trace)
    LAST_RESULTS = res

    out_full = np.empty((K, AO, BO), dtype=np.float32)
    for core in range(NCORES):
        out_full[core * P : (core + 1) * P] = res.results[core]["out"]

    # [K, AO, BO] -> [AO, BO, R, C]
    return np.ascontiguousarray(
        out_full.reshape(R, C, AO, BO).transpose(2, 3, 0, 1)
    )


if __name__ == "__main__":
    rng = np.random.default_rng(0)
    xs = rng.standard_normal((A, B, R, C), dtype=np.float32)
    out = kernel(
        xs,
        rng.standard_normal(K).astype(np.float32) * 0.1,
        rng.standard_normal(K).astype(np.float32) * 0.1,
        rng.standard_normal((K, KS, KS)).astype(np.float32),
        rng.standard_normal(K).astype(np.float32),
    )
    print(out.shape, out.dtype)
